# revision 11
# baseline (speedup 1.0000x reference)
"""Bilinear 2x upsample (16,3,512,512)->(16,3,1024,1024) on 8 trn2 NeuronCores.

Exact 2x bilinear: src = dst * 0.5, so
  out[2r, 2c]     = x[r, c]
  out[2r, 2c+1]   = 0.5*x[r, c]   + 0.5*x[r, c+1]   (clamped at c=511)
  out[2r+1, *]    = 0.5*row(2r,*) + 0.5*row(2r+2,*) (clamped at r=511)

Memory-bound problem: the kernel runs end-to-end in fp16 to halve HBM
traffic (16.5 MB/core vs 33 MB in f32). Input is quantized to fp16 on the
host; the device computes fp16 and stores fp16; the host upcasts the
result to f32. Max abs error vs the f32 reference is ~2e-3 on unit-scale
data (~3 ulp fp16) — orders of magnitude inside the 2e-2 gate.

Sharding: pure data parallel, 2 images (= 6 512x512 planes) per core.

Per-core layout: each plane is loaded as t5[128, 5, 514] fp16 with input
row r = 4p + b (partition p, block b; block 4 = overlap row 4p+4, clamped)
and two pad columns (512 = dup of 511 for the right-edge clamp, 513 = pad
so the 514-elem block stride stays 4B-aligned for DVE packed modes).
The overlapped+clamped layout is pre-gathered on the host so the load is
a single [128, 5140B-contiguous] DMA (partition-strided or single-row
loads measured 10-40x slower).

Engine split per plane (cycles est):
  ACT: th = 0.5*t5            (2.43us)   ACT: h even cols = t5 (2.43us, strided)
  DVE: h odd cols = th[c]+th[c+1]  (2.72us @1x, strided write)
  DVE: vs = h[b]+h[b+1]            (2.19us @2x)   DVE: vs *= 0.5 (1.13us @4x)
ACT ~4.9us, DVE ~6.0us per plane; DMA ~7.7us per plane -> DMA-bound.
"""

import sys

if "/opt/trn_rl_repo" not in sys.path:
    sys.path.insert(0, "/opt/trn_rl_repo")

import numpy as np

N_CORES = 8
N, C, HI, WI = 16, 3, 512, 512
HO, WO = 1024, 1024
PLANES = (N // N_CORES) * C  # 6 planes per core
P = 128
B = HI // P  # 4 row-blocks per partition
B5 = B + 1  # + 1 overlap block (row 4p+4)
WPAD = WI + 2  # 512 data cols + dup col (right clamp) + align pad

_cached = {}


def _split_excess_waits(nc, max_waits=1):
    """Hoist excess sem waits into no-ops so each instruction carries <=max_waits.

    The walrus build in this container rejects instructions carrying more
    sync-wait commands than the ISA encoding slot count ("Too many sync wait
    commands", e.g. TPB_CTRL holds 1). Tile's scheduler attaches one wait per
    producer proc to a single instruction through an unchecked path. Waiting on
    a chain of same-engine no-ops immediately before the instruction is
    semantically identical (the engine stream is sequential), so move the
    excess waits there.
    """
    import concourse.mybir as mybir

    for f in nc.m.functions:
        for bb in f.blocks:
            insts = bb.instructions
            if not any(
                i.sync_info is not None and len(i.sync_info.on_wait) > max_waits
                for i in insts
            ):
                continue
            new = []
            for inst in insts:
                si = inst.sync_info
                if si is not None and len(si.on_wait) > max_waits:
                    waits = list(si.on_wait)
                    for w in waits[max_waits:]:
                        nop = mybir.InstNoOp(
                            name=nc.get_next_instruction_name(),
                            engine=inst.engine,
                            sync_info=mybir.SyncInfo(on_wait=[w], on_update=[]),
                            bass_nofuse=True,
                        )
                        nc.register_instruction(nop, overwrite=True)
                        new.append(nop)
                    inst.sync_info = mybir.SyncInfo(
                        on_wait=waits[:max_waits], on_update=list(si.on_update)
                    )
                new.append(inst)
            bb.instructions = new


def _build_module(reps=1, bufs=4):
    import concourse.bass as bass
    import concourse.mybir as mybir
    import concourse.tile as tile

    f16 = mybir.dt.float16
    nc = bass.Bass()
    # x is the host-pre-gathered tile layout: [plane, partition, 5*514] fp16
    # with x[pl, p, b*514 + w] = image[pl, min(4p+b, 511), min(w, 511)].
    x = nc.dram_tensor("x", [PLANES, P, B5 * WPAD], f16, kind="ExternalInput")
    # Even (h) and odd (v) output rows land in separate contiguous buffers:
    # each store is one fully-contiguous 1MB region (8KB per partition), the
    # ideal HBM write pattern. The host interleaves rows during fp16->f32.
    outh = nc.dram_tensor("outh", [PLANES, HI, WO], f16, kind="ExternalOutput")
    outv = nc.dram_tensor("outv", [PLANES, HI, WO], f16, kind="ExternalOutput")

    with tile.TileContext(nc) as tc:
        with tc.tile_pool(name="pool", bufs=bufs) as pool:
            for pl in [p for _ in range(reps) for p in range(PLANES)]:
                # ---- load t5[p, b, w] = x[pl, min(4p+b, 511), min(w, 511)]
                # Loads + store-h go on the SP HWDGE ring (pure DMA dispatch,
                # no compute coupling), store-v on the gpsimd SWDGE ring: two
                # independent descriptor queues so a store blocked on compute
                # rarely head-of-line-blocks the next plane's load.
                t5 = pool.tile([P, B5, WPAD], f16)
                nc.sync.dma_start(
                    t5[:], x[:][pl].rearrange("p (b w) -> p b w", b=B5)
                )

                # ---- th = 0.5 * t5 (cols 0..512: data + dup col)
                # Ops are split into block halves so the first store can fire
                # before the whole plane's compute is done (shrinks pipeline
                # head/tail; Tile tracks deps at AP granularity).
                th = pool.tile([P, B5, WPAD], f16)
                nc.scalar.mul(th[:, 0:3, 0 : WI + 1], t5[:, 0:3, 0 : WI + 1], 0.5)
                nc.scalar.mul(th[:, 3:5, 0 : WI + 1], t5[:, 3:5, 0 : WI + 1], 0.5)

                # ---- horizontal interpolation -> h5[p, b, 2w interleaved]
                h5 = pool.tile([P, B5, WO], f16)
                nc.vector.tensor_add(
                    h5[:, 0:3, 1:WO:2], th[:, 0:3, 0:WI], th[:, 0:3, 1 : WI + 1]
                )
                nc.scalar.copy(h5[:, 0:2, 0:WO:2], t5[:, 0:2, 0:WI])
                nc.vector.tensor_add(
                    h5[:, 3:5, 1:WO:2], th[:, 3:5, 0:WI], th[:, 3:5, 1 : WI + 1]
                )
                nc.scalar.copy(h5[:, 2:5, 0:WO:2], t5[:, 2:5, 0:WI])

                # ---- vertical: odd out row 2(4p+b)+1 = 0.5*(h(4p+b)+h(4p+b+1))
                vs = pool.tile([P, B, WO], f16)
                nc.vector.tensor_add(vs[:, 0:2], h5[:, 0:2, :], h5[:, 1:3, :])
                nc.vector.tensor_scalar_mul(vs[:, 0:2], vs[:, 0:2], 0.5)
                nc.vector.tensor_add(vs[:, 2:4], h5[:, 2:4, :], h5[:, 3:5, :])
                nc.vector.tensor_scalar_mul(vs[:, 2:4], vs[:, 2:4], 0.5)

                # ---- stores: outh row 4p+b = out row 2(4p+b); outv likewise +1
                dsth = outh[:][pl].rearrange("(p b) w -> p b w", b=B)
                dstv = outv[:][pl].rearrange("(p b) w -> p b w", b=B)
                nc.sync.dma_start(dsth[:, 0:2], h5[:, 0:2, :])
                nc.gpsimd.dma_start(dstv[:, 0:2], vs[:, 0:2])
                nc.sync.dma_start(dsth[:, 2:4], h5[:, 2:4, :])
                nc.gpsimd.dma_start(dstv[:, 2:4], vs[:, 2:4])

    _split_excess_waits(nc)
    nc.finalize()
    return nc


def _get_module():
    if "nc" not in _cached:
        _cached["nc"] = _build_module()
    return _cached["nc"]


_ROW_IDX = np.minimum(
    4 * np.arange(P)[:, None] + np.arange(B5)[None, :], HI - 1
)  # [128, 5] source row per (partition, block)
_COL_IDX = np.minimum(np.arange(WPAD), WI - 1)  # [514] dup col 511 twice + pad


def _prep(planes):
    """fp16 [n_planes, 512, 512] image planes -> [n_planes, 128, 2570] layout."""
    g = planes[:, _ROW_IDX, :][..., _COL_IDX]  # [n, 128, 5, 514]
    return np.ascontiguousarray(g.reshape(planes.shape[0], P, B5 * WPAD))


def kernel(x, target_height=1024, target_width=1024):
    from concourse.bass_utils import run_bass_kernel_spmd

    assert int(target_height) == HO and int(target_width) == WO
    x = np.asarray(x, dtype=np.float32)
    assert x.shape == (N, C, HI, WI)
    xh = x.astype(np.float16)
    xg = _prep(xh.reshape(N * C, HI, WI))  # [48, 128, 2570] fp16

    nc = _get_module()
    per_core = N // N_CORES
    in_maps = [{"x": xg[i * PLANES : (i + 1) * PLANES]} for i in range(N_CORES)]
    res = run_bass_kernel_spmd(nc, in_maps, core_ids=list(range(N_CORES)))
    out = np.empty((N, C, HO, WO), np.float32)
    for i, r in enumerate(res.results):
        sl = out[i * per_core : (i + 1) * per_core]
        sl[:, :, 0::2, :] = r["outh"].reshape(per_core, C, HI, WO)
        sl[:, :, 1::2, :] = r["outv"].reshape(per_core, C, HI, WO)
    return out
